# revision 41
# baseline (speedup 1.0000x reference)
"""Trainium2 Bass kernel for nn_Attention_59785944760577 (sparse_attention).

reference math per batch sample (B=8 sharded one-per-NeuronCore):
  s[t]   = w2 . tanh(x[t] @ W1 + b1) + b2
  e[t]   = exp(s[t])            (softmax shift cancels in the num/den ratio)
  ctx[t] = cumsum_t(e * x) / cumsum_t(e)

Single software-pipelined loop over pairs of 128-row tiles (all matmul
traffic bf16, PSUM fp32 accumulation):
  - host supplies x in BOTH layouts as bf16 (natural [t,d] and transposed
    [d,t]) -> no PE transposes and half the input DMA of fp32.
  - pair stage q: h = tanh(xT @ W1) via bank-interleaved accumulating
    matmuls; s = sum_e h*w2 (DVE STT accum); e = exp(s+b2);
    Ue = u128 * e and ze = basis_k * e (DVE per-partition scales) fold the
    softmax weights into matmul stationaries -> no e*x elementwise pass.
  - tile totals T_k = ze_k^T x accumulate into ONE stacked PSUM bank
    [32,512] (basis-matmuls) -> no cross-partition copies, no serial
    carry chain.
  - per 4-tile group: den prefixes for the whole group in one [128,4]
    matmul slice; den carries via one tiny DVE mult + one bf16 matmul;
    one DVE reciprocal slice.
  - lag-6 stage: pN = Ue^T x (local prefix) + zbc_m^T totals (carry
    broadcast, bf16) accumulated into the same bank; out = pN * r with
    the scale split ACT/DVE; bf16 store (host upcasts to fp32).
The scan is causal, so output tiles stream out while later tiles are
still in the forward pass -- no phase barrier, PE stays HAM-warm.
"""
import json
from contextlib import ExitStack

import numpy as np

import concourse.bass as bass
import concourse.tile as tile
from concourse import mybir
from concourse.bass_utils import run_bass_kernel_spmd
from concourse.vector_clock import ScopedClock

F32 = mybir.dt.float32
BF16 = mybir.dt.bfloat16
F32R = mybir.dt.float32r
AF = mybir.ActivationFunctionType
ALU = mybir.AluOpType

B, T, D = 8, 4096, 512
P = 128
NT = T // P  # 32 tiles of 128 rows
NP = NT // 2  # 16 pairs
NG = 8  # DMA groups of 512 rows
N_CORES = 8


# --- workarounds for this walrus build: at most ONE semaphore wait per
# instruction.  (a) TileContext's exit drain batches one wait per live sem —
# emit one single-wait drain each instead.  (b) Tile's stage-1B wait
# assignment can put 2+ waits on ordinary instructions; split those in the
# serialized BIR JSON by inserting single-wait NoOps before the instruction.
def _patched_drain_and_barrier(self, tick_clock, wait_clock):
    nc = self.nc
    drain_inst = nc.sync.drain()
    wait_clock.add_sem_waits(
        drain_inst.ins, ScopedClock({None: tick_clock.global_clock})
    )
    si = drain_inst.ins.sync_info
    if si is not None and si.on_wait and len(si.on_wait) > 1:
        waits = list(si.on_wait)
        drain_inst.ins.sync_info = mybir.SyncInfo(
            on_wait=waits[:1], on_update=list(si.on_update)
        )
        for w in waits[1:]:
            extra = nc.sync.drain()
            extra.ins.sync_info = mybir.SyncInfo(on_wait=[w], on_update=[])
    nc.all_engine_barrier()
    assert self.sems is not None
    popped = nc._tile_sem_poison_stack.pop()
    assert popped is self._sem_poison
    nc.clear_and_free_semaphores(list(self.sems.allocated().values()))
    nc.all_engine_barrier()


def _split_multiwait_json(data: bytes) -> bytes:
    d = json.loads(data)
    changed = False
    for fn in d.get("functions", []):
        for bb in fn.get("blocks", []):
            new_insts = []
            for inst in bb.get("instructions", []):
                si = inst.get("sync_info")
                waits = si.get("on_wait") if si else None
                if waits and len(waits) > 1:
                    for k, w in enumerate(waits[:-1]):
                        new_insts.append(
                            {
                                "debug": inst.get("debug", 0),
                                "engine": inst["engine"],
                                "ins": [],
                                "outs": [],
                                "name": f"{inst['name']}-ws{k}",
                                "opcode": "NoOp",
                                "sync_info": {"on_update": [], "on_wait": [w]},
                            }
                        )
                    si["on_wait"] = [waits[-1]]
                    changed = True
                new_insts.append(inst)
            if changed:
                bb["instructions"] = new_insts
    return json.dumps(d).encode() if changed else data


def _install_patches():
    if not getattr(tile.TileContext, "_drain_patched", False):
        tile.TileContext._drain_and_barrier = _patched_drain_and_barrier
        tile.TileContext._drain_patched = True
    if not getattr(bass.Bass, "_json_waitsplit_patched", False):
        orig = bass.Bass.to_json_bytes

        def to_json_bytes(self):
            return _split_multiwait_json(orig(self))

        bass.Bass.to_json_bytes = to_json_bytes
        bass.Bass._json_waitsplit_patched = True


def build_nc(b2: float = 0.0):
    _install_patches()
    nc = bass.Bass()
    xn_d = nc.dram_tensor("xn", [T, D], BF16, kind="ExternalInput")
    xt_d = nc.dram_tensor("xt", [D, T], BF16, kind="ExternalInput")
    w1_d = nc.dram_tensor("w1", [D, D], BF16, kind="ExternalInput")
    w2r_d = nc.dram_tensor("w2r", [P, D], BF16, kind="ExternalInput")
    u128_d = nc.dram_tensor("u128", [P, P], BF16, kind="ExternalInput")
    u32s_d = nc.dram_tensor("u32s", [32, 32], BF16, kind="ExternalInput")
    onesb_d = nc.dram_tensor("onesb", [32, P], BF16, kind="ExternalInput")
    z_d = nc.dram_tensor("zbasis", [P, NT * 32], BF16, kind="ExternalInput")
    zbc_d = nc.dram_tensor("zbc", [32, NT * P], BF16, kind="ExternalInput")
    out_d = nc.dram_tensor("out", [T, D], BF16, kind="ExternalOutput")

    with tile.TileContext(nc) as tc, ExitStack() as ctx:
        consts = ctx.enter_context(tc.tile_pool(name="consts", bufs=1))
        xtp = ctx.enter_context(tc.tile_pool(name="xt", bufs=1))
        xnp = ctx.enter_context(tc.tile_pool(name="xn", bufs=1))
        hpool = ctx.enter_context(tc.tile_pool(name="h", bufs=4))
        spool = ctx.enter_context(tc.tile_pool(name="s", bufs=4))
        mpool = ctx.enter_context(tc.tile_pool(name="misc", bufs=1))
        obpool = ctx.enter_context(tc.tile_pool(name="ob", bufs=3))
        # PSUM (8 banks): HN 5 (h then num) + stackP 1 + stackD 1 + dall 1
        psHN = ctx.enter_context(tc.tile_pool(name="psHN", bufs=5, space="PSUM"))
        psSt = ctx.enter_context(tc.tile_pool(name="psSt", bufs=1, space="PSUM"))
        psStD = ctx.enter_context(tc.tile_pool(name="psStD", bufs=1, space="PSUM"))
        psDall = ctx.enter_context(tc.tile_pool(name="psDall", bufs=1, space="PSUM"))

        # x + w1 first (they gate compute); scan consts later
        w1_sb = consts.tile([P, 4, D], BF16, tag="w1")  # [d_in, c, e]
        nc.sync.dma_start(w1_sb[:], w1_d[:].rearrange("(c p) e -> p c e", p=P))
        xt_sb = xtp.tile([P, 4, T], BF16)  # [d%128, d//128, t]
        xn_sb = xnp.tile([P, NT, D], BF16)  # [t%128, t//128, d]
        w2r_sb = consts.tile([P, D], BF16, tag="w2r")
        z_sb = consts.tile([P, NT * 32], BF16, tag="z")
        u128_sb = consts.tile([P, P], BF16, tag="u128")
        u32s_sb = consts.tile([32, 32], BF16, tag="u32s")
        onesb_sb = consts.tile([32, P], BF16, tag="onesb")
        zbc_sb = consts.tile([32, NT * P], BF16, tag="zbc")
        for g in range(NG):
            sl = slice(512 * g, 512 * (g + 1))
            if g == 0:
                for hh in range(2):
                    sh = slice(256 * hh, 256 * (hh + 1))
                    nc.sync.dma_start(
                        xt_sb[:, :, sh],
                        xt_d[:, sh].rearrange("(c p) t -> p c t", p=P),
                    )
            else:
                nc.sync.dma_start(
                    xt_sb[:, :, sl], xt_d[:, sl].rearrange("(c p) t -> p c t", p=P)
                )
            nc.sync.dma_start(
                xn_sb[:, 4 * g : 4 * (g + 1), :],
                xn_d[sl, :].rearrange("(m p) d -> p m d", p=P),
            )
            if g == 0:
                nc.sync.dma_start(w2r_sb[:], w2r_d[:])
            elif g == 1:
                nc.sync.dma_start(z_sb[:], z_d[:])
            elif g == 2:
                nc.sync.dma_start(u128_sb[:], u128_d[:])
            elif g == 3:
                nc.sync.dma_start(u32s_sb[:], u32s_d[:])
                nc.sync.dma_start(onesb_sb[:], onesb_d[:])
                nc.sync.dma_start(zbc_sb[:], zbc_d[:])
        b2_sb = consts.tile([P, 1], F32, tag="b2")
        nc.vector.memset(b2_sb[:], float(b2))

        ecols = mpool.tile([P, NT], BF16, tag="ecols")
        stack32 = mpool.tile([32, D], BF16, tag="stack32")
        nc.vector.memset(stack32[:], 0.0)
        r32 = mpool.tile([32, 32], BF16, tag="r32")
        rall = mpool.tile([P, NT], F32, tag="rall")
        ues = mpool.tile([P, NT, P], BF16, tag="ues")

        stackP = psSt.tile([32, D], F32)
        stackDt = psStD.tile([32, 1], F32)
        stackD = stackDt[:]
        dallsd = psDall.tile([P, NT], F32)
        pDall = dallsd[:]
        scols = {}
        ecolfs = {}
        zes = {}
        hs = {}

        # single fully-pipelined loop over pairs of 128-row tiles.
        # stages per pair q: W1@q, tanh+STT@q+1, exp+Ue+ze@q+2, T@q+3,
        # per-group scan@2g+4, U+carry+scale+store@q+6.
        for it in range(NP + 7):
            if it < NP:
                a, b = 2 * it, 2 * it + 1
                pHa = psHN.tile([P, D], F32, name="pHa", tag="psHN")
                pHb = psHN.tile([P, D], F32, name="pHb", tag="psHN")
                for c in range(4):
                    nc.tensor.matmul(
                        pHa[:],
                        xt_sb[:, c, P * a : P * (a + 1)],
                        w1_sb[:, c, :],
                        start=(c == 0),
                        stop=(c == 3),
                    )
                    nc.tensor.matmul(
                        pHb[:],
                        xt_sb[:, c, P * b : P * (b + 1)],
                        w1_sb[:, c, :],
                        start=(c == 0),
                        stop=(c == 3),
                    )
                hs[a] = pHa
                hs[b] = pHb
            q = it - 3  # tile totals via basis matmuls (PE)
            if 0 <= q < NP:
                for k in (2 * q, 2 * q + 1):
                    zk = zes[k][:]
                    nc.tensor.matmul(
                        stackP[:],
                        zk,
                        xn_sb[:, k, :],
                        start=(k == 0),
                        stop=(k == NT - 1),
                    )
                    nc.tensor.matmul(
                        stackD,
                        zk,
                        u128_sb[:, P - 1 : P],
                        start=(k == 0),
                        stop=(k == NT - 1),
                        skip_group_check=True,
                    )
            if it >= 4 and it % 2 == 0 and (it - 4) // 2 < NG:
                g = (it - 4) // 2  # per-group scan: copies + den prefix/carry
                gs = slice(4 * g, 4 * (g + 1))
                nc.vector.tensor_copy(stack32[:], stackP[:])
                nc.tensor.matmul(
                    pDall[:, gs], u128_sb[:], ecols[:, gs],
                    start=True, stop=False, skip_group_check=True,
                )
                nc.vector.tensor_scalar_mul(r32[:, gs], u32s_sb[:, gs], stackD)
                nc.tensor.matmul(
                    pDall[:, gs], onesb_sb[:], r32[:, gs],
                    start=False, stop=True, skip_group_check=True,
                )
                nc.vector.reciprocal(rall[:, gs], pDall[:, gs])
            q = it - 6  # local prefix + carry broadcast + scale + store
            if 0 <= q < NP:
                ta, tb = 2 * q, 2 * q + 1
                pNa = psHN.tile([P, D], F32, name="pNa", tag="psHN")
                pNb = psHN.tile([P, D], F32, name="pNb", tag="psHN")
                nc.tensor.matmul(
                    pNa[:], ues[:, ta, :], xn_sb[:, ta, :], start=True, stop=False
                )
                nc.tensor.matmul(
                    pNb[:], ues[:, tb, :], xn_sb[:, tb, :], start=True, stop=False
                )
                nc.tensor.matmul(
                    pNa[:], zbc_sb[:, P * ta : P * (ta + 1)], stack32[:],
                    start=False, stop=True,
                )
                nc.tensor.matmul(
                    pNb[:], zbc_sb[:, P * tb : P * (tb + 1)], stack32[:],
                    start=False, stop=True,
                )
                ob = obpool.tile([P, 2, D], BF16, name="ob", tag="ob")
                nc.scalar.activation(
                    ob[:, 0, :], pNa[:], AF.Copy, scale=rall[:, ta : ta + 1]
                )
                nc.vector.tensor_scalar_mul(
                    ob[:, 1, :], pNb[:], rall[:, tb : tb + 1]
                )
                nc.sync.dma_start(
                    out_d[256 * q : 256 * (q + 1), :].rearrange(
                        "(m p) d -> p m d", p=P
                    ),
                    ob[:],
                )
            q = it - 1  # tanh (ACT) then s-dot (DVE)
            if 0 <= q < NP:
                for k in (2 * q, 2 * q + 1):
                    h = hpool.tile([P, D], BF16, name="h", tag="h")
                    nc.scalar.activation(h[:], hs[k][:], AF.Tanh)
                    hs[k] = h
                for k in (2 * q, 2 * q + 1):
                    scr = hpool.tile([P, D], BF16, name="scr", tag="scr")
                    scol = spool.tile([P, 1], F32, name="scol", tag="scol")
                    nc.vector.scalar_tensor_tensor(
                        scr[:], hs[k][:], 1.0, w2r_sb[:], ALU.mult, ALU.mult,
                        accum_out=scol[:],
                    )
                    scols[k] = scol
            q = it - 2  # exp (ACT); combined Ue|ze product (DVE)
            if 0 <= q < NP:
                for k in (2 * q, 2 * q + 1):
                    ecol = spool.tile([P, 1], F32, name="ecol", tag="ecol")
                    nc.scalar.activation(
                        ecol[:], scols[k][:], AF.Exp, bias=b2_sb[:, 0:1]
                    )
                    ecolfs[k] = ecol
                for k in (2 * q, 2 * q + 1):
                    nc.vector.tensor_copy(ecols[:, k : k + 1], ecolfs[k][:])
                    nc.vector.tensor_scalar_mul(
                        ues[:, k, :], u128_sb[:], ecolfs[k][:]
                    )
                    ze = spool.tile([P, 32], BF16, name="ze", tag="ze")
                    nc.vector.tensor_scalar_mul(
                        ze[:], z_sb[:, 32 * k : 32 * (k + 1)], ecolfs[k][:]
                    )
                    zes[k] = ze
    return nc


_NC_CACHE: dict[float, object] = {}


def _get_nc(b2: float):
    if b2 not in _NC_CACHE:
        _NC_CACHE[b2] = build_nc(b2)
    return _NC_CACHE[b2]


def _in_maps(x, W1, b1, w2):
    import ml_dtypes

    bf = ml_dtypes.bfloat16
    u128 = np.triu(np.ones((P, P), dtype=np.float32)).astype(bf)
    u32s = np.triu(np.ones((32, 32), dtype=np.float32), k=1).astype(bf)
    onesb = np.ones((32, P), dtype=np.float32).astype(bf)
    z = np.tile(np.eye(NT, dtype=np.float32), (P, 1)).reshape(P, NT * 32).astype(bf)
    zbc = np.repeat(
        np.triu(np.ones((32, 32), dtype=np.float32), k=1), P, axis=1
    ).astype(bf)
    w1_bf = np.ascontiguousarray(W1, dtype=bf)
    w2r_bf = np.ascontiguousarray(
        np.broadcast_to(np.asarray(w2, dtype=bf), (P, D))
    )
    assert not np.any(np.asarray(b1)), "b1 != 0 not supported by this build"
    maps = []
    for b in range(B):
        xb = np.ascontiguousarray(x[b], dtype=bf)
        maps.append(
            {
                "xn": xb,
                "xt": np.ascontiguousarray(xb.T),
                "w1": w1_bf,
                "w2r": w2r_bf,
                "u128": u128,
                "u32s": u32s,
                "onesb": onesb,
                "zbasis": z,
                "zbc": zbc,
            }
        )
    return maps


def kernel(x, W1, b1, w2, b2, _trace=False, _trace_cores=None):
    x = np.asarray(x)
    assert x.shape == (B, T, D), x.shape
    nc = _get_nc(float(np.asarray(b2)))
    res = run_bass_kernel_spmd(
        nc,
        _in_maps(x, W1, b1, w2),
        core_ids=list(range(N_CORES)),
        trace=_trace,
        trace_cores=_trace_cores,
    )
    out = np.stack(
        [np.asarray(res.results[i]["out"], dtype=np.float32) for i in range(N_CORES)],
        axis=0,
    )
    if _trace:
        return out, res
    return out


# revision 42
# speedup vs baseline: 1.1857x; 1.1857x over previous
"""Trainium2 Bass kernel for nn_Attention_59785944760577 (sparse_attention).

reference math per batch sample (B=8 sharded one-per-NeuronCore):
  s[t]   = w2 . tanh(x[t] @ W1 + b1) + b2
  e[t]   = exp(s[t])            (softmax shift cancels in the num/den ratio)
  ctx[t] = cumsum_t(e * x) / cumsum_t(e)

Single software-pipelined loop over pairs of 128-row tiles (all matmul
traffic bf16, PSUM fp32 accumulation):
  - host supplies x in BOTH layouts as bf16 (natural [t,d] and transposed
    [d,t]) -> no PE transposes and half the input DMA of fp32.
  - pair stage q: h = tanh(xT @ W1) via bank-interleaved accumulating
    matmuls; s = sum_e h*w2 (DVE STT accum); e = exp(s+b2);
    Ue = u128 * e and ze = basis_k * e (DVE per-partition scales) fold the
    softmax weights into matmul stationaries -> no e*x elementwise pass.
  - tile totals T_k = ze_k^T x accumulate into ONE stacked PSUM bank
    [32,512] (basis-matmuls) -> no cross-partition copies, no serial
    carry chain.
  - per 4-tile group: den prefixes for the whole group in one [128,4]
    matmul slice; den carries via one tiny DVE mult + one bf16 matmul;
    one DVE reciprocal slice.
  - lag-6 stage: pN = Ue^T x (local prefix) + zbc_m^T totals (carry
    broadcast, bf16) accumulated into the same bank; out = pN * r with
    the scale split ACT/DVE; bf16 store (host upcasts to fp32).
The scan is causal, so output tiles stream out while later tiles are
still in the forward pass -- no phase barrier, PE stays HAM-warm.
"""
import json
from contextlib import ExitStack

import numpy as np

import concourse.bass as bass
import concourse.tile as tile
from concourse import mybir
from concourse.bass_utils import run_bass_kernel_spmd
from concourse.vector_clock import ScopedClock

F32 = mybir.dt.float32
BF16 = mybir.dt.bfloat16
F32R = mybir.dt.float32r
AF = mybir.ActivationFunctionType
ALU = mybir.AluOpType

B, T, D = 8, 4096, 512
P = 128
NT = T // P  # 32 tiles of 128 rows
NP = NT // 2  # 16 pairs
NG = 8  # DMA groups of 512 rows
N_CORES = 8


# --- workarounds for this walrus build: at most ONE semaphore wait per
# instruction.  (a) TileContext's exit drain batches one wait per live sem —
# emit one single-wait drain each instead.  (b) Tile's stage-1B wait
# assignment can put 2+ waits on ordinary instructions; split those in the
# serialized BIR JSON by inserting single-wait NoOps before the instruction.
def _patched_drain_and_barrier(self, tick_clock, wait_clock):
    nc = self.nc
    drain_inst = nc.sync.drain()
    wait_clock.add_sem_waits(
        drain_inst.ins, ScopedClock({None: tick_clock.global_clock})
    )
    si = drain_inst.ins.sync_info
    if si is not None and si.on_wait and len(si.on_wait) > 1:
        waits = list(si.on_wait)
        drain_inst.ins.sync_info = mybir.SyncInfo(
            on_wait=waits[:1], on_update=list(si.on_update)
        )
        for w in waits[1:]:
            extra = nc.sync.drain()
            extra.ins.sync_info = mybir.SyncInfo(on_wait=[w], on_update=[])
    nc.all_engine_barrier()
    assert self.sems is not None
    popped = nc._tile_sem_poison_stack.pop()
    assert popped is self._sem_poison
    nc.clear_and_free_semaphores(list(self.sems.allocated().values()))
    nc.all_engine_barrier()


def _split_multiwait_json(data: bytes) -> bytes:
    d = json.loads(data)
    changed = False
    for fn in d.get("functions", []):
        for bb in fn.get("blocks", []):
            new_insts = []
            for inst in bb.get("instructions", []):
                si = inst.get("sync_info")
                waits = si.get("on_wait") if si else None
                if waits and len(waits) > 1:
                    for k, w in enumerate(waits[:-1]):
                        new_insts.append(
                            {
                                "debug": inst.get("debug", 0),
                                "engine": inst["engine"],
                                "ins": [],
                                "outs": [],
                                "name": f"{inst['name']}-ws{k}",
                                "opcode": "NoOp",
                                "sync_info": {"on_update": [], "on_wait": [w]},
                            }
                        )
                    si["on_wait"] = [waits[-1]]
                    changed = True
                new_insts.append(inst)
            if changed:
                bb["instructions"] = new_insts
    return json.dumps(d).encode() if changed else data


def _install_patches():
    if not getattr(tile.TileContext, "_drain_patched", False):
        tile.TileContext._drain_and_barrier = _patched_drain_and_barrier
        tile.TileContext._drain_patched = True
    if not getattr(bass.Bass, "_json_waitsplit_patched", False):
        orig = bass.Bass.to_json_bytes

        def to_json_bytes(self):
            return _split_multiwait_json(orig(self))

        bass.Bass.to_json_bytes = to_json_bytes
        bass.Bass._json_waitsplit_patched = True


def build_nc(b2: float = 0.0):
    _install_patches()
    nc = bass.Bass()
    xn_d = nc.dram_tensor("xn", [T, D], BF16, kind="ExternalInput")
    xt_d = nc.dram_tensor("xt", [D, T], BF16, kind="ExternalInput")
    w1_d = nc.dram_tensor("w1", [D, D], BF16, kind="ExternalInput")
    w2r_d = nc.dram_tensor("w2r", [P, D], BF16, kind="ExternalInput")
    u128_d = nc.dram_tensor("u128", [P, P], BF16, kind="ExternalInput")
    u32s_d = nc.dram_tensor("u32s", [32, 32], BF16, kind="ExternalInput")
    onesb_d = nc.dram_tensor("onesb", [32, P], BF16, kind="ExternalInput")
    z_d = nc.dram_tensor("zbasis", [P, NT * 32], BF16, kind="ExternalInput")
    zbc_d = nc.dram_tensor("zbc", [32, NT * P], BF16, kind="ExternalInput")
    out_d = nc.dram_tensor("out", [T, D], BF16, kind="ExternalOutput")

    with tile.TileContext(nc) as tc, ExitStack() as ctx:
        consts = ctx.enter_context(tc.tile_pool(name="consts", bufs=1))
        xtp = ctx.enter_context(tc.tile_pool(name="xt", bufs=1))
        xnp = ctx.enter_context(tc.tile_pool(name="xn", bufs=1))
        hpool = ctx.enter_context(tc.tile_pool(name="h", bufs=4))
        spool = ctx.enter_context(tc.tile_pool(name="s", bufs=4))
        mpool = ctx.enter_context(tc.tile_pool(name="misc", bufs=1))
        obpool = ctx.enter_context(tc.tile_pool(name="ob", bufs=3))
        # PSUM (8 banks): HN 5 (h then num) + stackP 1 + stackD 1 + dall 1
        psHN = ctx.enter_context(tc.tile_pool(name="psHN", bufs=5, space="PSUM"))
        psSt = ctx.enter_context(tc.tile_pool(name="psSt", bufs=1, space="PSUM"))
        psStD = ctx.enter_context(tc.tile_pool(name="psStD", bufs=1, space="PSUM"))
        psDall = ctx.enter_context(tc.tile_pool(name="psDall", bufs=1, space="PSUM"))

        # x + w1 first (they gate compute); scan consts later
        w1_sb = consts.tile([P, 4, D], BF16, tag="w1")  # [d_in, c, e]
        nc.sync.dma_start(w1_sb[:], w1_d[:].rearrange("(c p) e -> p c e", p=P))
        xt_sb = xtp.tile([P, 4, T], BF16)  # [d%128, d//128, t]
        xn_sb = xnp.tile([P, NT, D], BF16)  # [t%128, t//128, d]
        w2r_sb = consts.tile([P, D], BF16, tag="w2r")
        z_sb = consts.tile([P, NT * 32], BF16, tag="z")
        u128_sb = consts.tile([P, P], BF16, tag="u128")
        u32s_sb = consts.tile([32, 32], BF16, tag="u32s")
        onesb_sb = consts.tile([32, P], BF16, tag="onesb")
        zbc_sb = consts.tile([32, NT * P], BF16, tag="zbc")
        for g in range(NG):
            sl = slice(512 * g, 512 * (g + 1))
            if g == 0:
                for hh in range(2):
                    sh = slice(256 * hh, 256 * (hh + 1))
                    nc.sync.dma_start(
                        xt_sb[:, :, sh],
                        xt_d[:, sh].rearrange("(c p) t -> p c t", p=P),
                    )
            else:
                nc.sync.dma_start(
                    xt_sb[:, :, sl], xt_d[:, sl].rearrange("(c p) t -> p c t", p=P)
                )
            nc.sync.dma_start(
                xn_sb[:, 4 * g : 4 * (g + 1), :],
                xn_d[sl, :].rearrange("(m p) d -> p m d", p=P),
            )
            if g == 0:
                nc.sync.dma_start(w2r_sb[:], w2r_d[:])
            elif g == 1:
                nc.sync.dma_start(z_sb[:], z_d[:])
            elif g == 2:
                nc.sync.dma_start(u128_sb[:], u128_d[:])
            elif g == 3:
                nc.sync.dma_start(u32s_sb[:], u32s_d[:])
                nc.sync.dma_start(onesb_sb[:], onesb_d[:])
                nc.sync.dma_start(zbc_sb[:], zbc_d[:])
        b2_sb = consts.tile([P, 1], F32, tag="b2")
        nc.vector.memset(b2_sb[:], float(b2))

        ecols = mpool.tile([P, NT], BF16, tag="ecols")
        stack32 = mpool.tile([32, D], BF16, tag="stack32")
        nc.vector.memset(stack32[:], 0.0)
        r32 = mpool.tile([32, 32], BF16, tag="r32")
        rall = mpool.tile([P, NT], F32, tag="rall")
        ues = mpool.tile([P, NT, P], BF16, tag="ues")

        stackP = psSt.tile([32, D], F32)
        stackDt = psStD.tile([32, 1], F32)
        stackD = stackDt[:]
        dallsd = psDall.tile([P, NT], F32)
        pDall = dallsd[:]
        scols = {}
        ecolfs = {}
        zes = {}
        hs = {}

        # single fully-pipelined loop over pairs of 128-row tiles.
        # stages per pair q: W1@q, tanh+STT@q+1, exp+Ue+ze@q+2, T@q+3,
        # per-group scan@2g+4, U+carry+scale+store@q+6.
        for it in range(NP + 7):
            q = it - 6  # local prefix + carry broadcast + scale + store
            if 0 <= q < NP:
                ta, tb = 2 * q, 2 * q + 1
                pNa = psHN.tile([P, D], F32, name="pNa", tag="psHN")
                pNb = psHN.tile([P, D], F32, name="pNb", tag="psHN")
                nc.tensor.matmul(
                    pNa[:], ues[:, ta, :], xn_sb[:, ta, :], start=True, stop=False
                )
                nc.tensor.matmul(
                    pNb[:], ues[:, tb, :], xn_sb[:, tb, :], start=True, stop=False
                )
                nc.tensor.matmul(
                    pNa[:], zbc_sb[:, P * ta : P * (ta + 1)], stack32[:],
                    start=False, stop=True,
                )
                nc.tensor.matmul(
                    pNb[:], zbc_sb[:, P * tb : P * (tb + 1)], stack32[:],
                    start=False, stop=True,
                )
                ob = obpool.tile([P, 2, D], BF16, name="ob", tag="ob")
                nc.scalar.activation(
                    ob[:, 0, :], pNa[:], AF.Copy, scale=rall[:, ta : ta + 1]
                )
                nc.vector.tensor_scalar_mul(
                    ob[:, 1, :], pNb[:], rall[:, tb : tb + 1]
                )
                nc.sync.dma_start(
                    out_d[256 * q : 256 * (q + 1), :].rearrange(
                        "(m p) d -> p m d", p=P
                    ),
                    ob[:],
                )
            if it < NP:
                a, b = 2 * it, 2 * it + 1
                pHa = psHN.tile([P, D], F32, name="pHa", tag="psHN")
                pHb = psHN.tile([P, D], F32, name="pHb", tag="psHN")
                for c in range(4):
                    nc.tensor.matmul(
                        pHa[:],
                        xt_sb[:, c, P * a : P * (a + 1)],
                        w1_sb[:, c, :],
                        start=(c == 0),
                        stop=(c == 3),
                    )
                    nc.tensor.matmul(
                        pHb[:],
                        xt_sb[:, c, P * b : P * (b + 1)],
                        w1_sb[:, c, :],
                        start=(c == 0),
                        stop=(c == 3),
                    )
                for k, pH in ((a, pHa), (b, pHb)):
                    h = hpool.tile([P, D], BF16, name="h", tag="h")
                    nc.scalar.activation(h[:], pH[:], AF.Tanh)
                    hs[k] = h
            q = it - 3  # tile totals via basis matmuls (PE)
            if 0 <= q < NP:
                for k in (2 * q, 2 * q + 1):
                    zk = zes[k][:]
                    nc.tensor.matmul(
                        stackP[:],
                        zk,
                        xn_sb[:, k, :],
                        start=(k == 0),
                        stop=(k == NT - 1),
                    )
                    nc.tensor.matmul(
                        stackD,
                        zk,
                        u128_sb[:, P - 1 : P],
                        start=(k == 0),
                        stop=(k == NT - 1),
                        skip_group_check=True,
                    )
            if it >= 4 and it % 2 == 0 and (it - 4) // 2 < NG:
                g = (it - 4) // 2  # per-group scan: copies + den prefix/carry
                gs = slice(4 * g, 4 * (g + 1))
                nc.vector.tensor_copy(stack32[:], stackP[:])
                nc.tensor.matmul(
                    pDall[:, gs], u128_sb[:], ecols[:, gs],
                    start=True, stop=False, skip_group_check=True,
                )
                nc.vector.tensor_scalar_mul(r32[:, gs], u32s_sb[:, gs], stackD)
                nc.tensor.matmul(
                    pDall[:, gs], onesb_sb[:], r32[:, gs],
                    start=False, stop=True, skip_group_check=True,
                )
                nc.vector.reciprocal(rall[:, gs], pDall[:, gs])
            q = it - 1  # s-dot (DVE)
            if 0 <= q < NP:
                for k in (2 * q, 2 * q + 1):
                    scr = hpool.tile([P, D], BF16, name="scr", tag="scr")
                    scol = spool.tile([P, 1], F32, name="scol", tag="scol")
                    nc.vector.scalar_tensor_tensor(
                        scr[:], hs[k][:], 1.0, w2r_sb[:], ALU.mult, ALU.mult,
                        accum_out=scol[:],
                    )
                    scols[k] = scol
            q = it - 2  # exp (ACT); combined Ue|ze product (DVE)
            if 0 <= q < NP:
                for k in (2 * q, 2 * q + 1):
                    ecol = spool.tile([P, 1], F32, name="ecol", tag="ecol")
                    nc.scalar.activation(
                        ecol[:], scols[k][:], AF.Exp, bias=b2_sb[:, 0:1]
                    )
                    ecolfs[k] = ecol
                for k in (2 * q, 2 * q + 1):
                    nc.vector.tensor_copy(ecols[:, k : k + 1], ecolfs[k][:])
                    nc.vector.tensor_scalar_mul(
                        ues[:, k, :], u128_sb[:], ecolfs[k][:]
                    )
                    ze = spool.tile([P, 32], BF16, name="ze", tag="ze")
                    nc.vector.tensor_scalar_mul(
                        ze[:], z_sb[:, 32 * k : 32 * (k + 1)], ecolfs[k][:]
                    )
                    zes[k] = ze
    return nc


_NC_CACHE: dict[float, object] = {}


def _get_nc(b2: float):
    if b2 not in _NC_CACHE:
        _NC_CACHE[b2] = build_nc(b2)
    return _NC_CACHE[b2]


def _in_maps(x, W1, b1, w2):
    import ml_dtypes

    bf = ml_dtypes.bfloat16
    u128 = np.triu(np.ones((P, P), dtype=np.float32)).astype(bf)
    u32s = np.triu(np.ones((32, 32), dtype=np.float32), k=1).astype(bf)
    onesb = np.ones((32, P), dtype=np.float32).astype(bf)
    z = np.tile(np.eye(NT, dtype=np.float32), (P, 1)).reshape(P, NT * 32).astype(bf)
    zbc = np.repeat(
        np.triu(np.ones((32, 32), dtype=np.float32), k=1), P, axis=1
    ).astype(bf)
    w1_bf = np.ascontiguousarray(W1, dtype=bf)
    w2r_bf = np.ascontiguousarray(
        np.broadcast_to(np.asarray(w2, dtype=bf), (P, D))
    )
    assert not np.any(np.asarray(b1)), "b1 != 0 not supported by this build"
    maps = []
    for b in range(B):
        xb = np.ascontiguousarray(x[b], dtype=bf)
        maps.append(
            {
                "xn": xb,
                "xt": np.ascontiguousarray(xb.T),
                "w1": w1_bf,
                "w2r": w2r_bf,
                "u128": u128,
                "u32s": u32s,
                "onesb": onesb,
                "zbasis": z,
                "zbc": zbc,
            }
        )
    return maps


def kernel(x, W1, b1, w2, b2, _trace=False, _trace_cores=None):
    x = np.asarray(x)
    assert x.shape == (B, T, D), x.shape
    nc = _get_nc(float(np.asarray(b2)))
    res = run_bass_kernel_spmd(
        nc,
        _in_maps(x, W1, b1, w2),
        core_ids=list(range(N_CORES)),
        trace=_trace,
        trace_cores=_trace_cores,
    )
    out = np.stack(
        [np.asarray(res.results[i]["out"], dtype=np.float32) for i in range(N_CORES)],
        axis=0,
    )
    if _trace:
        return out, res
    return out


# revision 43
# speedup vs baseline: 1.2026x; 1.0143x over previous
"""Trainium2 Bass kernel for nn_Attention_59785944760577 (sparse_attention).

reference math per batch sample (B=8 sharded one-per-NeuronCore):
  s[t]   = w2 . tanh(x[t] @ W1 + b1) + b2
  e[t]   = exp(s[t])            (softmax shift cancels in the num/den ratio)
  ctx[t] = cumsum_t(e * x) / cumsum_t(e)

Single software-pipelined loop over pairs of 128-row tiles (all matmul
traffic bf16, PSUM fp32 accumulation):
  - host supplies x in BOTH layouts as bf16 (natural [t,d] and transposed
    [d,t]) -> no PE transposes and half the input DMA of fp32.
  - pair stage q: h = tanh(xT @ W1) via bank-interleaved accumulating
    matmuls, with tanh in the same iteration so PSUM banks recycle
    fastest; s = sum_e h*w2 (DVE STT accum); e = exp(s+b2);
    Ue = u128 * e and ze = basis_k * e (DVE per-partition scales) fold the
    softmax weights into matmul stationaries -> no e*x elementwise pass.
  - tile totals T_k = ze_k^T x accumulate into ONE stacked PSUM bank
    [32,512] (basis-matmuls) -> no cross-partition copies, no serial
    carry chain.
  - per 4-tile group: den prefixes for the whole group in one [128,4]
    matmul slice; den carries via one tiny DVE mult + one bf16 matmul;
    one DVE reciprocal slice.
  - lag-6 stage: pN = Ue^T x (local prefix) + zbc_m^T totals (carry
    broadcast, bf16) accumulated into the same bank; out = pN * r with
    the scale split ACT/DVE; bf16 store (host upcasts to fp32).
The scan is causal, so output tiles stream out while later tiles are
still in the forward pass -- no phase barrier, PE stays HAM-warm.
"""
import json
from contextlib import ExitStack

import numpy as np

import concourse.bass as bass
import concourse.tile as tile
from concourse import mybir
from concourse.bass_utils import run_bass_kernel_spmd
from concourse.vector_clock import ScopedClock

F32 = mybir.dt.float32
BF16 = mybir.dt.bfloat16
F32R = mybir.dt.float32r
AF = mybir.ActivationFunctionType
ALU = mybir.AluOpType

B, T, D = 8, 4096, 512
P = 128
NT = T // P  # 32 tiles of 128 rows
NP = NT // 2  # 16 pairs
NG = 8  # DMA groups of 512 rows
N_CORES = 8


# --- workarounds for this walrus build: at most ONE semaphore wait per
# instruction.  (a) TileContext's exit drain batches one wait per live sem —
# emit one single-wait drain each instead.  (b) Tile's stage-1B wait
# assignment can put 2+ waits on ordinary instructions; split those in the
# serialized BIR JSON by inserting single-wait NoOps before the instruction.
def _patched_drain_and_barrier(self, tick_clock, wait_clock):
    nc = self.nc
    drain_inst = nc.sync.drain()
    wait_clock.add_sem_waits(
        drain_inst.ins, ScopedClock({None: tick_clock.global_clock})
    )
    si = drain_inst.ins.sync_info
    if si is not None and si.on_wait and len(si.on_wait) > 1:
        waits = list(si.on_wait)
        drain_inst.ins.sync_info = mybir.SyncInfo(
            on_wait=waits[:1], on_update=list(si.on_update)
        )
        for w in waits[1:]:
            extra = nc.sync.drain()
            extra.ins.sync_info = mybir.SyncInfo(on_wait=[w], on_update=[])
    nc.all_engine_barrier()
    assert self.sems is not None
    popped = nc._tile_sem_poison_stack.pop()
    assert popped is self._sem_poison
    nc.clear_and_free_semaphores(list(self.sems.allocated().values()))
    nc.all_engine_barrier()


def _split_multiwait_json(data: bytes) -> bytes:
    d = json.loads(data)
    changed = False
    for fn in d.get("functions", []):
        for bb in fn.get("blocks", []):
            new_insts = []
            for inst in bb.get("instructions", []):
                si = inst.get("sync_info")
                waits = si.get("on_wait") if si else None
                if waits and len(waits) > 1:
                    for k, w in enumerate(waits[:-1]):
                        new_insts.append(
                            {
                                "debug": inst.get("debug", 0),
                                "engine": inst["engine"],
                                "ins": [],
                                "outs": [],
                                "name": f"{inst['name']}-ws{k}",
                                "opcode": "NoOp",
                                "sync_info": {"on_update": [], "on_wait": [w]},
                            }
                        )
                    si["on_wait"] = [waits[-1]]
                    changed = True
                new_insts.append(inst)
            if changed:
                bb["instructions"] = new_insts
    return json.dumps(d).encode() if changed else data


def _install_patches():
    if not getattr(tile.TileContext, "_drain_patched", False):
        tile.TileContext._drain_and_barrier = _patched_drain_and_barrier
        tile.TileContext._drain_patched = True
    if not getattr(bass.Bass, "_json_waitsplit_patched", False):
        orig = bass.Bass.to_json_bytes

        def to_json_bytes(self):
            return _split_multiwait_json(orig(self))

        bass.Bass.to_json_bytes = to_json_bytes
        bass.Bass._json_waitsplit_patched = True


def build_nc(b2: float = 0.0):
    _install_patches()
    nc = bass.Bass()
    xn_d = nc.dram_tensor("xn", [T, D], BF16, kind="ExternalInput")
    xt_d = nc.dram_tensor("xt", [D, T], BF16, kind="ExternalInput")
    w1_d = nc.dram_tensor("w1", [D, D], BF16, kind="ExternalInput")
    w2r_d = nc.dram_tensor("w2r", [P, D], BF16, kind="ExternalInput")
    u128_d = nc.dram_tensor("u128", [P, P], BF16, kind="ExternalInput")
    u32s_d = nc.dram_tensor("u32s", [32, 32], BF16, kind="ExternalInput")
    onesb_d = nc.dram_tensor("onesb", [32, P], BF16, kind="ExternalInput")
    z_d = nc.dram_tensor("zbasis", [P, NT * 32], BF16, kind="ExternalInput")
    zbc_d = nc.dram_tensor("zbc", [32, NT * P], BF16, kind="ExternalInput")
    out_d = nc.dram_tensor("out", [T, D], BF16, kind="ExternalOutput")

    with tile.TileContext(nc) as tc, ExitStack() as ctx:
        consts = ctx.enter_context(tc.tile_pool(name="consts", bufs=1))
        xtp = ctx.enter_context(tc.tile_pool(name="xt", bufs=1))
        xnp = ctx.enter_context(tc.tile_pool(name="xn", bufs=1))
        hpool = ctx.enter_context(tc.tile_pool(name="h", bufs=4))
        spool = ctx.enter_context(tc.tile_pool(name="s", bufs=4))
        mpool = ctx.enter_context(tc.tile_pool(name="misc", bufs=1))
        obpool = ctx.enter_context(tc.tile_pool(name="ob", bufs=3))
        # PSUM (8 banks): HN 5 (h then num) + stackP 1 + stackD 1 + dall 1
        psHN = ctx.enter_context(tc.tile_pool(name="psHN", bufs=5, space="PSUM"))
        psSt = ctx.enter_context(tc.tile_pool(name="psSt", bufs=1, space="PSUM"))
        psStD = ctx.enter_context(tc.tile_pool(name="psStD", bufs=1, space="PSUM"))
        psDall = ctx.enter_context(tc.tile_pool(name="psDall", bufs=1, space="PSUM"))

        # x + w1 first (they gate compute); scan consts later
        w1_sb = consts.tile([P, 4, D], BF16, tag="w1")  # [d_in, c, e]
        nc.sync.dma_start(w1_sb[:], w1_d[:].rearrange("(c p) e -> p c e", p=P))
        xt_sb = xtp.tile([P, 4, T], BF16)  # [d%128, d//128, t]
        xn_sb = xnp.tile([P, NT, D], BF16)  # [t%128, t//128, d]
        w2r_sb = consts.tile([P, D], BF16, tag="w2r")
        z_sb = consts.tile([P, NT * 32], BF16, tag="z")
        u128_sb = consts.tile([P, P], BF16, tag="u128")
        u32s_sb = consts.tile([32, 32], BF16, tag="u32s")
        onesb_sb = consts.tile([32, P], BF16, tag="onesb")
        zbc_sb = consts.tile([32, NT * P], BF16, tag="zbc")
        for g in range(NG):
            sl = slice(512 * g, 512 * (g + 1))
            if g == 0:
                for hh in range(2):
                    sh = slice(256 * hh, 256 * (hh + 1))
                    nc.sync.dma_start(
                        xt_sb[:, :, sh],
                        xt_d[:, sh].rearrange("(c p) t -> p c t", p=P),
                    )
            else:
                nc.sync.dma_start(
                    xt_sb[:, :, sl], xt_d[:, sl].rearrange("(c p) t -> p c t", p=P)
                )
            nc.sync.dma_start(
                xn_sb[:, 4 * g : 4 * (g + 1), :],
                xn_d[sl, :].rearrange("(m p) d -> p m d", p=P),
            )
            if g == 0:
                nc.sync.dma_start(w2r_sb[:], w2r_d[:])
            elif g == 1:
                nc.sync.dma_start(z_sb[:], z_d[:])
            elif g == 2:
                nc.sync.dma_start(u128_sb[:], u128_d[:])
            elif g == 3:
                nc.sync.dma_start(u32s_sb[:], u32s_d[:])
                nc.sync.dma_start(onesb_sb[:], onesb_d[:])
                nc.sync.dma_start(zbc_sb[:], zbc_d[:])
        b2_sb = consts.tile([P, 1], F32, tag="b2")
        nc.vector.memset(b2_sb[:], float(b2))

        ecols = mpool.tile([P, NT], BF16, tag="ecols")
        stack32 = mpool.tile([32, D], BF16, tag="stack32")
        nc.vector.memset(stack32[:], 0.0)
        r32 = mpool.tile([32, 32], BF16, tag="r32")
        rall = mpool.tile([P, NT], F32, tag="rall")
        ues = mpool.tile([P, NT, P], BF16, tag="ues")

        stackP = psSt.tile([32, D], F32)
        stackDt = psStD.tile([32, 1], F32)
        stackD = stackDt[:]
        dallsd = psDall.tile([P, NT], F32)
        pDall = dallsd[:]
        scols = {}
        ecolfs = {}
        zes = {}
        hs = {}

        # single fully-pipelined loop over pairs of 128-row tiles.
        # stages per pair q: W1@q, tanh+STT@q+1, exp+Ue+ze@q+2, T@q+3,
        # per-group scan@2g+4, U+carry+scale+store@q+6.
        for it in range(NP + 7):
            q = it - 6  # local prefix + carry broadcast + scale + store
            if 0 <= q < NP:
                ta, tb = 2 * q, 2 * q + 1
                pNa = psHN.tile([P, D], F32, name="pNa", tag="psHN")
                pNb = psHN.tile([P, D], F32, name="pNb", tag="psHN")
                nc.tensor.matmul(
                    pNa[:], ues[:, ta, :], xn_sb[:, ta, :], start=True, stop=False
                )
                nc.tensor.matmul(
                    pNb[:], ues[:, tb, :], xn_sb[:, tb, :], start=True, stop=False
                )
                nc.tensor.matmul(
                    pNa[:], zbc_sb[:, P * ta : P * (ta + 1)], stack32[:],
                    start=False, stop=True,
                )
                nc.tensor.matmul(
                    pNb[:], zbc_sb[:, P * tb : P * (tb + 1)], stack32[:],
                    start=False, stop=True,
                )
                ob = obpool.tile([P, 2, D], BF16, name="ob", tag="ob")
                nc.scalar.activation(
                    ob[:, 0, :], pNa[:], AF.Copy, scale=rall[:, ta : ta + 1]
                )
                nc.vector.tensor_scalar_mul(
                    ob[:, 1, :], pNb[:], rall[:, tb : tb + 1]
                )
                nc.sync.dma_start(
                    out_d[256 * q : 256 * (q + 1), :].rearrange(
                        "(m p) d -> p m d", p=P
                    ),
                    ob[:],
                )
            if it < NP:
                a, b = 2 * it, 2 * it + 1
                pHa = psHN.tile([P, D], F32, name="pHa", tag="psHN")
                pHb = psHN.tile([P, D], F32, name="pHb", tag="psHN")
                for c in range(4):
                    nc.tensor.matmul(
                        pHa[:],
                        xt_sb[:, c, P * a : P * (a + 1)],
                        w1_sb[:, c, :],
                        start=(c == 0),
                        stop=(c == 3),
                    )
                    nc.tensor.matmul(
                        pHb[:],
                        xt_sb[:, c, P * b : P * (b + 1)],
                        w1_sb[:, c, :],
                        start=(c == 0),
                        stop=(c == 3),
                    )
                for k, pH in ((a, pHa), (b, pHb)):
                    h = hpool.tile([P, D], BF16, name="h", tag="h")
                    nc.scalar.activation(h[:], pH[:], AF.Tanh)
                    hs[k] = h
            q = it - 3  # tile totals via basis matmuls (PE)
            if 0 <= q < NP:
                for k in (2 * q, 2 * q + 1):
                    zk = zes[k][:]
                    nc.tensor.matmul(
                        stackP[:],
                        zk,
                        xn_sb[:, k, :],
                        start=(k == 0),
                        stop=(k == NT - 1),
                    )
                    nc.tensor.matmul(
                        stackD,
                        zk,
                        u128_sb[:, P - 1 : P],
                        start=(k == 0),
                        stop=(k == NT - 1),
                        skip_group_check=True,
                    )
            if it >= 4 and it % 2 == 0 and (it - 4) // 2 < NG:
                g = (it - 4) // 2  # per-group scan: copies + den prefix/carry
                gs = slice(4 * g, 4 * (g + 1))
                nc.vector.tensor_copy(stack32[:], stackP[:])
                nc.tensor.matmul(
                    pDall[:, gs], u128_sb[:], ecols[:, gs],
                    start=True, stop=False, skip_group_check=True,
                )
                nc.vector.tensor_scalar_mul(r32[:, gs], u32s_sb[:, gs], stackD)
                nc.tensor.matmul(
                    pDall[:, gs], onesb_sb[:], r32[:, gs],
                    start=False, stop=True, skip_group_check=True,
                )
                nc.vector.reciprocal(rall[:, gs], pDall[:, gs])
            q = it - 1  # s-dot (DVE)
            if 0 <= q < NP:
                for k in (2 * q, 2 * q + 1):
                    scr = hpool.tile([P, D], BF16, name="scr", tag="scr")
                    scol = spool.tile([P, 1], F32, name="scol", tag="scol")
                    nc.vector.scalar_tensor_tensor(
                        scr[:], hs[k][:], 1.0, w2r_sb[:], ALU.mult, ALU.mult,
                        accum_out=scol[:],
                    )
                    scols[k] = scol
            q = it - 2  # exp (ACT); combined Ue|ze product (DVE)
            if 0 <= q < NP:
                for k in (2 * q, 2 * q + 1):
                    ecol = spool.tile([P, 1], F32, name="ecol", tag="ecol")
                    nc.scalar.activation(
                        ecol[:], scols[k][:], AF.Exp, bias=b2_sb[:, 0:1]
                    )
                    ecolfs[k] = ecol
                for k in (2 * q, 2 * q + 1):
                    nc.vector.tensor_copy(ecols[:, k : k + 1], ecolfs[k][:])
                    nc.vector.tensor_scalar_mul(
                        ues[:, k, :], u128_sb[:], ecolfs[k][:]
                    )
                    ze = spool.tile([P, 32], BF16, name="ze", tag="ze")
                    nc.vector.tensor_scalar_mul(
                        ze[:], z_sb[:, 32 * k : 32 * (k + 1)], ecolfs[k][:]
                    )
                    zes[k] = ze
    return nc


_NC_CACHE: dict[float, object] = {}


def _get_nc(b2: float):
    if b2 not in _NC_CACHE:
        _NC_CACHE[b2] = build_nc(b2)
    return _NC_CACHE[b2]


def _in_maps(x, W1, b1, w2):
    import ml_dtypes

    bf = ml_dtypes.bfloat16
    u128 = np.triu(np.ones((P, P), dtype=np.float32)).astype(bf)
    u32s = np.triu(np.ones((32, 32), dtype=np.float32), k=1).astype(bf)
    onesb = np.ones((32, P), dtype=np.float32).astype(bf)
    z = np.tile(np.eye(NT, dtype=np.float32), (P, 1)).reshape(P, NT * 32).astype(bf)
    zbc = np.repeat(
        np.triu(np.ones((32, 32), dtype=np.float32), k=1), P, axis=1
    ).astype(bf)
    w1_bf = np.ascontiguousarray(W1, dtype=bf)
    w2r_bf = np.ascontiguousarray(
        np.broadcast_to(np.asarray(w2, dtype=bf), (P, D))
    )
    assert not np.any(np.asarray(b1)), "b1 != 0 not supported by this build"
    maps = []
    for b in range(B):
        xb = np.ascontiguousarray(x[b], dtype=bf)
        maps.append(
            {
                "xn": xb,
                "xt": np.ascontiguousarray(xb.T),
                "w1": w1_bf,
                "w2r": w2r_bf,
                "u128": u128,
                "u32s": u32s,
                "onesb": onesb,
                "zbasis": z,
                "zbc": zbc,
            }
        )
    return maps


def kernel(x, W1, b1, w2, b2, _trace=False, _trace_cores=None):
    x = np.asarray(x)
    assert x.shape == (B, T, D), x.shape
    nc = _get_nc(float(np.asarray(b2)))
    res = run_bass_kernel_spmd(
        nc,
        _in_maps(x, W1, b1, w2),
        core_ids=list(range(N_CORES)),
        trace=_trace,
        trace_cores=_trace_cores,
    )
    out = np.stack(
        [np.asarray(res.results[i]["out"], dtype=np.float32) for i in range(N_CORES)],
        axis=0,
    )
    if _trace:
        return out, res
    return out
